# revision 1
# baseline (speedup 1.0000x reference)
"""Trainium2 Bass kernel for nn_DCNModel_12816182411985.

Model: DCN — shared deep MLP (1024->500->200->200 with relu) + 2-task
cross-net + sigmoid heads on concat([emb, d3]) @ Wl.

Key algebraic collapse: the cross-net iteration
    emb_{j+1} = s * emb_j * cw[i,j] + cb[i,j] + x      (s = sum(x, axis=1))
is affine per (batch, feature), so emb3 = x * P_i(s) + Q_i(s) with cubic
polynomials in s whose coefficients are per-feature vectors.  Hence

  emb3 @ w_emb = (x@w) + s*(x@(cw2*w)) + s^2*(x@(cw1*cw2*w)) + s^3*(x@(cw0*cw1*cw2*w))
                 + s*(cb1*cw2 . w) + s^2*(cb0*cw1*cw2 . w) + (cb2 . w)

All x-projections (8 columns incl. a ones-column producing s) are folded
into the big x @ W1 matmul as extra output columns.  The per-batch cubic
combine is done with a few DVE row ops + one tiny selection matmul that
also accumulates d3 @ Wl_d3.

Sharding: data-parallel batch split across 8 cores; weights replicated.
Matmuls run in float32r (1-pass FP22 reads, fp32 PSUM accumulate).
Orientation: features on partitions, batch on the free axis; x is
transposed on-chip via PE transpose-mode.
"""

import os
import numpy as np

B, DIM = 16384, 1024
H1, H2, H3 = 500, 200, 200
NCORES = 8
BPC = B // NCORES        # 2048 batch rows per core
NTILE = 512              # batch columns per tile
NT = BPC // NTILE        # 4 column tiles per core
NCH = NTILE // 128       # 4 batch chunks of 128 per column tile
KF = DIM // 128          # 8 feature k-tiles

# d1 row layout (after column permutation of W1):
#   rows   0:480  -> W1 cols 0:480
#   rows 480:488  -> tail block [s, y1_0, y2_0, y3_0, y1_1, y2_1, y3_1, y0]
#   rows 488:508  -> W1 cols 480:500
#   rows 508:512  -> zero pad
# In m-tile 3 (partitions 0..127 <-> rows 384..511) the tail block sits at
# partitions 96..103 (32-aligned, as required for matmul tile_position).

_CACHE = {}
LAST_RESULTS = None  # BassKernelResults of the most recent run (for test.py)


def _build_nc(stage="full"):
    import concourse.bacc as bacc
    import concourse.mybir as mybir
    import concourse.tile as tile
    from concourse.masks import make_identity

    f32 = mybir.dt.float32
    f32r = mybir.dt.float32r
    AF = mybir.ActivationFunctionType

    nc = bacc.Bacc("TRN2", target_bir_lowering=False, debug=False)

    x_d = nc.dram_tensor("x_shard", [BPC, DIM], f32, kind="ExternalInput")
    w1_d = nc.dram_tensor("w1aug", [DIM, 512], f32r, kind="ExternalInput")
    w2_d = nc.dram_tensor("w2aug", [512, H2], f32r, kind="ExternalInput")
    w3_d = nc.dram_tensor("w3m", [H2, H3], f32r, kind="ExternalInput")
    wd3_d = nc.dram_tensor("wd3dup", [H3, 2], f32r, kind="ExternalInput")
    sel_d = nc.dram_tensor("sel", [128, 2], f32r, kind="ExternalInput")
    b1_d = nc.dram_tensor("b1aug", [128, 4], f32, kind="ExternalInput")
    b2_d = nc.dram_tensor("b2arr", [100, 2], f32, kind="ExternalInput")
    b3_d = nc.dram_tensor("b3arr", [100, 2], f32, kind="ExternalInput")
    sigb_d = nc.dram_tensor("sigb", [2, 1], f32, kind="ExternalInput")
    mask_d = nc.dram_tensor("tailmask", [128, 6], f32, kind="ExternalInput")
    ones_d = nc.dram_tensor("onesrow", [1, NTILE], f32r, kind="ExternalInput")
    out_d = nc.dram_tensor("preds", [2, BPC], f32, kind="ExternalOutput")
    dbg_d = None
    if stage != "full":
        dbg_d = nc.dram_tensor("dbg", [128, NTILE], f32, kind="ExternalOutput")

    from contextlib import ExitStack
    with tile.TileContext(nc) as tc, ExitStack() as stack:
        # ---------- constants / weights (resident for the whole kernel) ----
        consts_pool = stack.enter_context(tc.tile_pool(name="consts", bufs=1))

        def single(shape, name, dtype=f32):
            return consts_pool.tile(shape, dtype, name=name, tag=name)

        ident = single([128, 128], "ident")
        make_identity(nc, ident)

        w1sb = []
        for f in range(KF):
            t = single([128, 512], f"w1sb{f}", f32r)
            nc.sync.dma_start(out=t, in_=w1_d[f * 128:(f + 1) * 128, :])
            w1sb.append(t)
        w2sb = []
        for k in range(4):
            t = single([128, H2], f"w2sb{k}", f32r)
            nc.sync.dma_start(out=t, in_=w2_d[k * 128:(k + 1) * 128, :])
            w2sb.append(t)
        w3sb = []
        for k in range(2):
            t = single([100, H3], f"w3sb{k}", f32r)
            nc.sync.dma_start(out=t, in_=w3_d[k * 100:(k + 1) * 100, :])
            w3sb.append(t)
        wd3sb = []
        for k in range(2):
            t = single([100, 2], f"wd3sb{k}", f32r)
            nc.sync.dma_start(out=t, in_=wd3_d[k * 100:(k + 1) * 100, :])
            wd3sb.append(t)
        selsb = single([128, 2], "selsb", f32r)
        nc.sync.dma_start(out=selsb, in_=sel_d[:, :])
        b1sb = single([128, 4], "b1sb")
        nc.sync.dma_start(out=b1sb, in_=b1_d[:, :])
        b2sb = single([100, 2], "b2sb")
        nc.sync.dma_start(out=b2sb, in_=b2_d[:, :])
        b3sb = single([100, 2], "b3sb")
        nc.sync.dma_start(out=b3sb, in_=b3_d[:, :])
        sigbsb = single([2, 1], "sigbsb")
        nc.sync.dma_start(out=sigbsb, in_=sigb_d[:, :])
        maskbuf = single([128, 6], "maskbuf")
        nc.sync.dma_start(out=maskbuf, in_=mask_d[:, :])
        ones8 = single([128, 8], "ones8", f32r)
        nc.sync.dma_start(out=ones8[96:97, :], in_=ones_d[0:1, 0:8])

        with (
            tc.tile_pool(name="xnat", bufs=6) as xnat_pool,
            tc.tile_pool(name="xT", bufs=12) as xt_pool,
            tc.tile_pool(name="d1p", bufs=6) as d1_pool,
            tc.tile_pool(name="d2p", bufs=3) as d2_pool,
            tc.tile_pool(name="d3p", bufs=3) as d3_pool,
            tc.tile_pool(name="osbp", bufs=2) as out_pool,
            tc.tile_pool(name="tmpp", bufs=2) as tmp_pool,
            tc.tile_pool(name="ptrans", bufs=2, space="PSUM") as ptrans,
            tc.tile_pool(name="pl1", bufs=2, space="PSUM") as pl1,
            tc.tile_pool(name="pl2", bufs=1, space="PSUM") as pl2,
            tc.tile_pool(name="pl3", bufs=1, space="PSUM") as pl3,
            tc.tile_pool(name="pP", bufs=1, space="PSUM") as pP_pool,
            tc.tile_pool(name="plog", bufs=1, space="PSUM") as plog_pool,
        ):
            for n in range(NT):
                base = n * NTILE

                # x chunks, natural (batch-major) layout
                xch = []
                for c in range(NCH):
                    t = xnat_pool.tile([128, DIM], f32, tag="xnat",
                                       name=f"xn{n}_{c}")
                    nc.sync.dma_start(
                        out=t, in_=x_d[base + c * 128: base + (c + 1) * 128, :])
                    xch.append(t)

                # transpose to feature-major via PE
                xt = []
                for f in range(KF):
                    pt = ptrans.tile([128, NTILE], f32, tag="pt",
                                     name=f"pt{n}_{f}")
                    for c in range(NCH):
                        nc.tensor.transpose(
                            pt[:, c * 128:(c + 1) * 128],
                            xch[c][:, f * 128:(f + 1) * 128], ident)
                    st = xt_pool.tile([128, NTILE], f32r, tag="xt",
                                      name=f"xt{n}_{f}")
                    nc.vector.tensor_copy(st, pt)
                    xt.append(st)
                if stage == "xt":
                    if n == 0:
                        nc.sync.dma_start(out=dbg_d[:, :], in_=xt[0].bitcast(f32))
                    continue

                # L1: d1 = relu(x @ W1aug + b1aug), tail rows copied raw
                d1 = []
                for m in range(4):
                    p1 = pl1.tile([128, NTILE], f32, tag="p1",
                                  name=f"p1_{n}_{m}")
                    for f in range(KF):
                        nc.tensor.matmul(
                            p1, w1sb[f][:, m * 128:(m + 1) * 128], xt[f],
                            start=(f == 0), stop=(f == KF - 1))
                    dt_ = d1_pool.tile([128, NTILE], f32r, tag="d1",
                                       name=f"d1_{n}_{m}")
                    nc.scalar.activation(out=dt_, in_=p1, func=AF.Relu,
                                         bias=b1sb[:, m:m + 1], scale=1.0)
                    if m == 3:
                        # overwrite tail rows with raw psum + c-constants
                        nc.vector.tensor_scalar_add(
                            dt_[96:104, :], p1[96:104, :], b1sb[96:104, 3:4])
                    d1.append(dt_)
                d13 = d1[3]
                if stage == "l1":
                    if n == 0:
                        nc.sync.dma_start(out=dbg_d[:, :], in_=d1[0].bitcast(f32))
                    continue

                # tail products: three rounds of T *= (mask_one + mask_s*s)
                # tail rows 96..103 = [s, y1_0, y2_0, y3_0, y1_1, y2_1, y3_1, y0]
                # psS rows 0:8 = s broadcast (one K=1 matmul per column tile)
                psS = pP_pool.tile([128, NTILE], f32, tag="pP", name=f"psS{n}")
                nc.tensor.matmul(psS[0:8, :], ones8[96:97, :], d13[96:97, :],
                                 start=True, stop=True, tile_position=(96, 0))

                def tail_round(j):
                    tmp = tmp_pool.tile([128, NTILE], f32, tag="tmp",
                                        name=f"tmp{n}_{j}")
                    nc.vector.tensor_scalar(
                        out=tmp[96:104, :], in0=psS[0:8, :],
                        scalar1=maskbuf[96:104, j:j + 1],
                        scalar2=maskbuf[96:104, 3 + j:4 + j],
                        op0=mybir.AluOpType.mult, op1=mybir.AluOpType.add)
                    nc.vector.tensor_mul(d13[96:104, :], d13[96:104, :],
                                         tmp[96:104, :])

                if stage != "notail":
                    tail_round(0)

                # L2: d2 = relu(d1 @ W2aug + b2)   (tail rows hit zero weights)
                d2 = []
                for m in range(2):
                    p2 = pl2.tile([100, NTILE], f32, tag="p2",
                                  name=f"p2_{n}_{m}")
                    for k in range(4):
                        nc.tensor.matmul(
                            p2, w2sb[k][:, m * 100:(m + 1) * 100], d1[k],
                            start=(k == 0), stop=(k == 3))
                    t2 = d2_pool.tile([100, NTILE], f32r, tag="d2",
                                      name=f"d2_{n}_{m}")
                    nc.scalar.activation(out=t2, in_=p2, func=AF.Relu,
                                         bias=b2sb[:, m:m + 1], scale=1.0)
                    d2.append(t2)
                if stage == "l2":
                    if n == 0:
                        nc.sync.dma_start(out=dbg_d[0:100, :], in_=d2[0].bitcast(f32))
                    continue

                if stage != "notail":
                    tail_round(1)

                # L3: d3 = relu(d2 @ W3 + b3)
                d3 = []
                for m in range(2):
                    p3 = pl3.tile([100, NTILE], f32, tag="p3",
                                  name=f"p3_{n}_{m}")
                    for k in range(2):
                        nc.tensor.matmul(
                            p3, w3sb[k][:, m * 100:(m + 1) * 100], d2[k],
                            start=(k == 0), stop=(k == 1))
                    t3 = d3_pool.tile([100, NTILE], f32r, tag="d3",
                                      name=f"d3_{n}_{m}")
                    nc.scalar.activation(out=t3, in_=p3, func=AF.Relu,
                                         bias=b3sb[:, m:m + 1], scale=1.0)
                    d3.append(t3)

                if stage != "notail":
                    tail_round(2)

                # logits: selection matmul over tail rows + d3 @ Wl_d3
                pl = plog_pool.tile([2, NTILE], f32, tag="plg", name=f"plog{n}")
                nc.tensor.matmul(pl, selsb[96:104, :], d13[96:104, :],
                                 start=True, stop=False,
                                 tile_position=(96, 0))
                nc.tensor.matmul(pl, wd3sb[0], d3[0],
                                 start=False, stop=False)
                nc.tensor.matmul(pl, wd3sb[1], d3[1],
                                 start=False, stop=True)

                osb = out_pool.tile([2, NTILE], f32, tag="osb", name=f"osb{n}")
                nc.scalar.activation(out=osb, in_=pl, func=AF.Sigmoid,
                                     bias=sigbsb, scale=1.0)
                nc.sync.dma_start(out=out_d[:, base:base + NTILE], in_=osb)

    nc.finalize()
    return nc


def _prep_host(W1, b1, W2, b2, W3, b3, Wl, bl, cw, cb):
    """Build the augmented/permuted parameter arrays."""
    W1 = np.asarray(W1, np.float32)
    b1 = np.asarray(b1, np.float32)
    W2 = np.asarray(W2, np.float32)
    b2 = np.asarray(b2, np.float32)
    W3 = np.asarray(W3, np.float32)
    b3 = np.asarray(b3, np.float32)
    Wl = np.asarray(Wl, np.float32)
    bl = np.asarray(bl, np.float32)
    cw = np.asarray(cw, np.float32)
    cb = np.asarray(cb, np.float32)

    w_emb = Wl[:DIM, 0]
    w_d3 = Wl[DIM:, 0]

    u = np.zeros((DIM, 8), np.float32)
    u[:, 0] = 1.0                      # s = x @ ones
    c1 = np.zeros(2, np.float32)
    c2 = np.zeros(2, np.float32)
    c0 = np.zeros(2, np.float32)
    for i in range(2):
        cw2 = cw[i, 2]
        cw12 = cw[i, 1] * cw2
        cw012 = cw[i, 0] * cw12
        u[:, 1 + 3 * i] = cw2 * w_emb
        u[:, 2 + 3 * i] = cw12 * w_emb
        u[:, 3 + 3 * i] = cw012 * w_emb
        c1[i] = float(np.dot(cb[i, 1] * cw2, w_emb))
        c2[i] = float(np.dot(cb[i, 0] * cw12, w_emb))
        c0[i] = float(np.dot(cb[i, 2], w_emb))
    u[:, 7] = w_emb                    # y0 = x @ w_emb

    w1aug = np.zeros((DIM, 512), np.float32)
    w1aug[:, 0:480] = W1[:, 0:480]
    w1aug[:, 480:488] = u
    w1aug[:, 488:508] = W1[:, 480:500]

    b1full = np.zeros(512, np.float32)
    b1full[0:480] = b1[0:480]
    b1full[480:488] = [0.0, c1[0], c2[0], 0.0, c1[1], c2[1], 0.0, 0.0]
    b1full[488:508] = b1[480:500]
    b1aug = np.ascontiguousarray(b1full.reshape(4, 128).T)

    w2aug = np.zeros((512, H2), np.float32)
    w2aug[0:480] = W2[0:480]
    w2aug[488:508] = W2[480:500]

    sel = np.zeros((128, 2), np.float32)
    sel[97:100, 0] = 1.0
    sel[103, 0] = 1.0
    sel[100:103, 1] = 1.0
    sel[103, 1] = 1.0

    wd3dup = np.ascontiguousarray(np.stack([w_d3, w_d3], axis=1))
    b2arr = np.ascontiguousarray(b2.reshape(2, 100).T)
    b3arr = np.ascontiguousarray(b3.reshape(2, 100).T)
    sigb = np.array([[c0[0] + bl[0]], [c0[1] + bl[0]]], np.float32)

    # tail-round masks: round j multiplies tail row r by
    # (mask_one[j][r] + mask_s[j][r]*s); after 3 rounds the rows
    # [s, y1_0, y2_0, y3_0, y1_1, y2_1, y3_1, y0] carry [s, y1*s, y2*s^2,
    # y3*s^3, ..., y0].  tailmask[:, j] = mask_s, tailmask[:, 3+j] = mask_one.
    tailmask = np.zeros((128, 6), np.float32)
    ones_masks = [[1, 0, 0, 0, 0, 0, 0, 1],
                  [1, 1, 0, 0, 1, 0, 0, 1],
                  [1, 1, 1, 0, 1, 1, 0, 1]]
    s_masks = [[0, 1, 1, 1, 1, 1, 1, 0],
               [0, 0, 1, 1, 0, 1, 1, 0],
               [0, 0, 0, 1, 0, 0, 1, 0]]
    for j in range(3):
        tailmask[96:104, j] = s_masks[j]
        tailmask[96:104, 3 + j] = ones_masks[j]

    return dict(w1aug=w1aug, w2aug=w2aug, w3m=np.ascontiguousarray(W3),
                wd3dup=wd3dup, sel=sel, b1aug=b1aug, b2arr=b2arr,
                b3arr=b3arr, sigb=sigb, tailmask=tailmask,
                onesrow=np.ones((1, NTILE), np.float32))


def kernel(x, show_index, st, W1, b1, W2, b2, W3, b3, Wl, bl, cw, cb):
    global LAST_RESULTS
    from concourse.bass_utils import run_bass_kernel_spmd

    x = np.ascontiguousarray(np.asarray(x, np.float32))
    params = _prep_host(W1, b1, W2, b2, W3, b3, Wl, bl, cw, cb)

    if "nc" not in _CACHE:
        _CACHE["nc"] = _build_nc()
    nc = _CACHE["nc"]

    core_ids = list(range(NCORES))
    in_maps = []
    for c in range(NCORES):
        m = {"x_shard": np.ascontiguousarray(x[c * BPC:(c + 1) * BPC])}
        m.update(params)
        in_maps.append(m)

    trace = bool(os.environ.get("DCN_TRACE"))
    res = run_bass_kernel_spmd(nc, in_maps, core_ids, trace=trace)
    LAST_RESULTS = res

    outs = [res.results[c]["preds"] for c in range(NCORES)]
    p0 = np.concatenate([o[0] for o in outs]).reshape(B, 1).astype(np.float32)
    p1 = np.concatenate([o[1] for o in outs]).reshape(B, 1).astype(np.float32)
    return (p0, p1)



# revision 35
# speedup vs baseline: 36397.9135x; 36397.9135x over previous
"""Trainium2 Bass kernel for nn_DCNModel_12816182411985.

Model: DCN — shared deep MLP (1024->500->200->200 with relu) + 2-task
cross-net + sigmoid heads on concat([emb, d3]) @ Wl.

Key algebraic collapse: the cross-net iteration
    emb_{j+1} = s * emb_j * cw[i,j] + cb[i,j] + x      (s = sum(x, axis=1))
is affine per (batch, feature), so emb3 = x * P_i(s) + Q_i(s) with cubic
polynomials in s whose coefficients are per-feature vectors.  Hence

  emb3 @ w_emb = (x@w) + s*(x@(cw2*w)) + s^2*(x@(cw1*cw2*w)) + s^3*(x@(cw0*cw1*cw2*w))
                 + s*(cb1*cw2 . w) + s^2*(cb0*cw1*cw2 . w) + (cb2 . w)

All x-projections (8 columns incl. a ones-column producing s) are folded
into the big x @ W1 matmul as extra output columns.  The per-batch cubic
combine is done with a few DVE row ops + one tiny selection matmul that
also accumulates d3 @ Wl_d3.

Sharding: data-parallel batch split across 8 cores; weights replicated.
x is shipped/stored as bf16 and transposed on load via the XBAR DMA
transpose (2-byte path), so the PE does no transposes at all; weights
stay f32r (1-pass FP22 reads) and accumulation is fp32 in PSUM.
Orientation: features on partitions, batch on the free axis.
"""

import numpy as np
import ml_dtypes

B, DIM = 16384, 1024
H1, H2, H3 = 500, 200, 200
NCORES = 8
BPC = B // NCORES        # 2048 batch rows per core
NTILE = 512              # batch columns per tile
NT = BPC // NTILE        # 4 column tiles per core
KF = DIM // 128          # 8 feature k-tiles

BF16 = ml_dtypes.bfloat16

# d1 row layout (after column permutation of W1):
#   rows   0:480  -> W1 cols 0:480
#   rows 480:488  -> tail block [s, y1_0, y2_0, y3_0, y1_1, y2_1, y3_1, y0]
#   rows 488:508  -> W1 cols 480:500
#   rows 508:512  -> zero pad
# In m-tile 3 (partitions 0..127 <-> rows 384..511) the tail block sits at
# partitions 96..103 (32-aligned, as required for matmul tile_position).

_CACHE = {}


def _build_nc(reps=1, loop=False, level=99, unroll=1):
    """level (profiling only): 1=xt 2=+l1mm 3=+act1 4=+tail 5=+l2 6=+l3
    7+=full. Production uses the default. loop wraps `unroll` python-
    unrolled passes in a hardware For_i loop (reps iterations)."""
    import concourse.bacc as bacc
    import concourse.mybir as mybir
    import concourse.tile as tile

    f32 = mybir.dt.float32
    f32r = mybir.dt.float32r
    bf16 = mybir.dt.bfloat16
    AF = mybir.ActivationFunctionType

    nc = bacc.Bacc("TRN2", target_bir_lowering=False, debug=False)

    # x, host-pretiled: block (n, f) = x^T[f*128:(f+1)*128, n*512:(n+1)*512]
    # stored contiguously at row (n*KF+f)*128 — every DMA load is one
    # contiguous 128 KB block.
    x_d = nc.dram_tensor("xt_shard", [NT * KF * 128, NTILE], bf16,
                         kind="ExternalInput")
    w1_d = nc.dram_tensor("w1aug", [DIM, 512], bf16, kind="ExternalInput")
    w2_d = nc.dram_tensor("w2aug", [512, H2], f32r, kind="ExternalInput")
    w3_d = nc.dram_tensor("w3m", [H2, H3], f32r, kind="ExternalInput")
    wd3_d = nc.dram_tensor("wd3dup", [H3, 2], f32r, kind="ExternalInput")
    sel_d = nc.dram_tensor("sel", [128, 2], f32r, kind="ExternalInput")
    b1_d = nc.dram_tensor("b1aug", [128, 4], f32, kind="ExternalInput")
    b2_d = nc.dram_tensor("b2arr", [100, 2], f32, kind="ExternalInput")
    b3_d = nc.dram_tensor("b3arr", [100, 2], f32, kind="ExternalInput")
    sigb_d = nc.dram_tensor("sigb", [2, 1], f32, kind="ExternalInput")
    mask_d = nc.dram_tensor("tailmask", [128, 6], f32, kind="ExternalInput")
    ones_d = nc.dram_tensor("onesrow", [1, NTILE], f32r, kind="ExternalInput")
    out_d = nc.dram_tensor("preds", [2, BPC], f32, kind="ExternalOutput")

    from contextlib import ExitStack
    with tile.TileContext(nc) as tc, ExitStack() as stack:
        # ---------- constants / weights (resident for the whole kernel) ----
        consts_pool = stack.enter_context(tc.tile_pool(name="consts", bufs=1))

        def single(shape, name, dtype=f32):
            return consts_pool.tile(shape, dtype, name=name, tag=name)

        w1sb = []
        for f in range(KF):
            t = single([128, 512], f"w1sb{f}", bf16)
            nc.sync.dma_start(out=t, in_=w1_d[f * 128:(f + 1) * 128, :])
            w1sb.append(t)
        w2sb = []
        for k in range(4):
            t = single([128, H2], f"w2sb{k}", f32r)
            nc.sync.dma_start(out=t, in_=w2_d[k * 128:(k + 1) * 128, :])
            w2sb.append(t)
        w3sb = []
        for k in range(2):
            t = single([100, H3], f"w3sb{k}", f32r)
            nc.sync.dma_start(out=t, in_=w3_d[k * 100:(k + 1) * 100, :])
            w3sb.append(t)
        wd3sb = []
        for k in range(2):
            t = single([100, 2], f"wd3sb{k}", f32r)
            nc.sync.dma_start(out=t, in_=wd3_d[k * 100:(k + 1) * 100, :])
            wd3sb.append(t)
        selsb = single([128, 2], "selsb", f32r)
        nc.sync.dma_start(out=selsb, in_=sel_d[:, :])
        b1sb = single([128, 4], "b1sb")
        nc.sync.dma_start(out=b1sb, in_=b1_d[:, :])
        b2sb = single([100, 2], "b2sb")
        nc.sync.dma_start(out=b2sb, in_=b2_d[:, :])
        b3sb = single([100, 2], "b3sb")
        nc.sync.dma_start(out=b3sb, in_=b3_d[:, :])
        sigbsb = single([2, 1], "sigbsb")
        nc.sync.dma_start(out=sigbsb, in_=sigb_d[:, :])
        maskbuf = single([128, 6], "maskbuf")
        nc.sync.dma_start(out=maskbuf, in_=mask_d[:, :])
        ones8 = single([128, 8], "ones8", f32r)
        nc.sync.dma_start(out=ones8[96:97, :], in_=ones_d[0:1, 0:8])

        uid = [0]

        def one_pass():
            # Stage-major PE order: the PE's in-order FIFO sees all L1
            # matmuls (all column tiles), then psS, L2, L3, logits. Each
            # stage's cross-engine inputs (Act relus, DVE relus/rounds)
            # were produced a full stage earlier, so the PE never blocks
            # mid-stream on another engine's latency. PSUM: pl1 gets 3
            # bufs so Act's drain (+semaphore latency) never stalls L1.
            uid[0] += 1
            u = uid[0]
            if level < 1:
                return

            have_tail = level >= 4
            tl = scp = None
            if level >= 3:
                tl = tl_pool.tile([128, BPC], f32r, tag="tl", name=f"tl{u}")
            if have_tail:
                scp = scp_pool.tile([128, BPC], f32, tag="scp", name=f"scp{u}")

            # stage 0 — x^T loads (one contiguous 1 MB DMA per column
            # tile, alternating HWDGE rings; they prefetch ahead)
            xts = []
            for n in range(NT):
                xtbig = xt_pool.tile([128, KF * NTILE], bf16, tag="xt",
                                     name=f"xt{u}_{n}")
                ring = nc.sync if n % 2 == 0 else nc.scalar
                ring.dma_start(
                    out=xtbig.rearrange("p (f c) -> p f c", f=KF),
                    in_=x_d[n * KF * 128:(n + 1) * KF * 128, :].rearrange(
                        "(f p) c -> p f c", p=128))
                xts.append(xtbig)
            if level < 2:
                return

            # stage 1 — L1: d1 = relu(x @ W1aug + b1aug)
            d1s = []
            for n in range(NT):
                base = n * NTILE
                xt = [xts[n][:, f * NTILE:(f + 1) * NTILE] for f in range(KF)]
                d1 = []
                for m in range(4):
                    p1 = pl1.tile([128, NTILE], f32, tag="p1",
                                  name=f"p1_{u}_{n}_{m}")
                    for f in range(KF):
                        nc.tensor.matmul(
                            p1, w1sb[f][:, m * 128:(m + 1) * 128], xt[f],
                            start=(f == 0), stop=(f == KF - 1))
                    if level < 3:
                        continue
                    dt_ = d1_pool.tile([128, NTILE], f32r, tag="d1",
                                       name=f"d1_{u}_{n}_{m}")
                    if m < 2:
                        # split the relu drain across Act and DVE so
                        # neither engine falls behind the PE's 1.75 us
                        # L1 group cadence (cross-engine latency ~1.5 us)
                        nc.scalar.activation(out=dt_, in_=p1, func=AF.Relu,
                                             bias=b1sb[:, m:m + 1], scale=1.0)
                    else:
                        nc.vector.tensor_scalar(
                            out=dt_, in0=p1, scalar1=b1sb[:, m:m + 1],
                            scalar2=0.0, op0=mybir.AluOpType.add,
                            op1=mybir.AluOpType.max)
                    if m == 3:
                        # raw tail rows (z + c-consts) -> pass-wide tile
                        nc.vector.tensor_scalar_add(
                            tl[96:104, base:base + NTILE], p1[96:104, :],
                            b1sb[96:104, 3:4])
                    d1.append(dt_)
                d1s.append(d1)
            if level < 3:
                return

            # stage 2 — s broadcast (K=1 matmuls) + SBUF copies
            if have_tail:
                for n in range(NT):
                    base = n * NTILE
                    psS = psh_pool.tile([128, NTILE], f32, tag="psh",
                                        name=f"psS{u}_{n}")
                    nc.tensor.matmul(psS[0:8, :], ones8[96:97, :],
                                     tl[96:97, base:base + NTILE],
                                     start=True, stop=True,
                                     tile_position=(96, 0))
                    nc.vector.tensor_copy(scp[0:8, base:base + NTILE],
                                          psS[0:8, :])
            if level < 5 and level >= 4:
                _merged_rounds(u, tl, scp)
                return
            if level < 5:
                return

            # stage 3 — L2: d2 = relu(d1 @ W2aug + b2), relu on DVE
            d2s = []
            for n in range(NT):
                d2 = []
                for m in range(2):
                    p2 = pl2.tile([100, NTILE], f32, tag="p2",
                                  name=f"p2_{u}_{n}_{m}")
                    for k in range(4):
                        nc.tensor.matmul(
                            p2, w2sb[k][:, m * 100:(m + 1) * 100], d1s[n][k],
                            start=(k == 0), stop=(k == 3))
                    t2 = d2_pool.tile([100, NTILE], f32r, tag="d2",
                                      name=f"d2_{u}_{n}_{m}")
                    # Act relu: ready well before the next pass's d1 relus,
                    # so no harmful FIFO HOL; keeps DVE free for the tail
                    nc.scalar.activation(out=t2, in_=p2, func=AF.Relu,
                                         bias=b2sb[:, m:m + 1], scale=1.0)
                    d2.append(t2)
                d2s.append(d2)
            if have_tail:
                _merged_rounds(u, tl, scp)
            if level < 6:
                return

            # stage 4 — L3: d3 = relu(d2 @ W3 + b3), relu on DVE
            d3s = []
            for n in range(NT):
                d3 = []
                for m in range(2):
                    p3 = pl3.tile([100, NTILE], f32, tag="p3",
                                  name=f"p3_{u}_{n}_{m}")
                    for k in range(2):
                        nc.tensor.matmul(
                            p3, w3sb[k][:, m * 100:(m + 1) * 100], d2s[n][k],
                            start=(k == 0), stop=(k == 1))
                    t3 = d3_pool.tile([100, NTILE], f32r, tag="d3",
                                      name=f"d3_{u}_{n}_{m}")
                    # Act relu: keeps DVE free to run the merged tail
                    # rounds during the L2/L3 stages (logits wait on them)
                    nc.scalar.activation(out=t3, in_=p3, func=AF.Relu,
                                         bias=b3sb[:, m:m + 1], scale=1.0)
                    d3.append(t3)
                d3s.append(d3)
            if level < 7:
                return

            # stage 5 — logits (tail selection + d3 @ Wl_d3), sigmoid, store
            for n in range(NT):
                base = n * NTILE
                pl = psh_pool.tile([128, NTILE], f32, tag="psh",
                                   name=f"plog{u}_{n}")
                nc.tensor.matmul(pl[0:2, :], selsb[96:104, :],
                                 tl[96:104, base:base + NTILE],
                                 start=True, stop=False,
                                 tile_position=(96, 0))
                nc.tensor.matmul(pl[0:2, :], wd3sb[0], d3s[n][0],
                                 start=False, stop=False)
                nc.tensor.matmul(pl[0:2, :], wd3sb[1], d3s[n][1],
                                 start=False, stop=True)

                # biased logits out via DVE; final sigmoid runs on the host
                # (keeps Act a pure d1-relu stream — an Act sigmoid here
                # would HOL-block the next pass's relus and serialize
                # passes through the pl1 PSUM pool)
                osb = out_pool.tile([2, NTILE], f32, tag="osb",
                                    name=f"osb{u}_{n}")
                nc.vector.tensor_scalar_add(osb, pl[0:2, :], sigbsb)
                # SWDGE (Pool) for the tiny stores: an HWDGE issue here
                # would wait on osb in the SP/Act FIFO and block the next
                # pass's x-load issues / relus
                nc.gpsimd.dma_start(out=out_d[:, base:base + NTILE], in_=osb)

        def _merged_rounds(u, tl, scp):
            # merged tail rounds over the whole pass on [8, BPC]:
            # 3 x (tmp = m_s*s + m_one; tl *= tmp); tail rows 96..103 =
            # [s, y1_0, y2_0, y3_0, y1_1, y2_1, y3_1, y0]
            for j in range(3):
                tmp = tmp_pool.tile([128, BPC], f32, tag="tmp",
                                    name=f"tmp{u}_{j}")
                nc.vector.tensor_scalar(
                    out=tmp[96:104, :], in0=scp[0:8, :],
                    scalar1=maskbuf[96:104, j:j + 1],
                    scalar2=maskbuf[96:104, 3 + j:4 + j],
                    op0=mybir.AluOpType.mult, op1=mybir.AluOpType.add)
                nc.vector.tensor_mul(tl[96:104, :], tl[96:104, :],
                                     tmp[96:104, :])

        with (
            tc.tile_pool(name="xT", bufs=7) as xt_pool,
            tc.tile_pool(name="d1p", bufs=6) as d1_pool,
            tc.tile_pool(name="d2p", bufs=4) as d2_pool,
            tc.tile_pool(name="d3p", bufs=8) as d3_pool,
            tc.tile_pool(name="osbp", bufs=2) as out_pool,
            tc.tile_pool(name="tmpp", bufs=2) as tmp_pool,
            tc.tile_pool(name="tlp", bufs=2) as tl_pool,
            tc.tile_pool(name="scpp", bufs=2) as scp_pool,
            tc.tile_pool(name="pl1", bufs=3, space="PSUM") as pl1,
            tc.tile_pool(name="pl2", bufs=2, space="PSUM") as pl2,
            tc.tile_pool(name="pl3", bufs=2, space="PSUM") as pl3,
            tc.tile_pool(name="psh", bufs=1, space="PSUM") as psh_pool,
        ):
            if loop and reps > 1:
                with tc.For_i(0, reps):
                    for _ in range(unroll):
                        one_pass()
            else:
                for _ in range(reps):
                    one_pass()

    nc.finalize()
    return nc


def _prep_host(W1, b1, W2, b2, W3, b3, Wl, bl, cw, cb):
    """Build the augmented/permuted parameter arrays."""
    W1 = np.asarray(W1, np.float32)
    b1 = np.asarray(b1, np.float32)
    W2 = np.asarray(W2, np.float32)
    b2 = np.asarray(b2, np.float32)
    W3 = np.asarray(W3, np.float32)
    b3 = np.asarray(b3, np.float32)
    Wl = np.asarray(Wl, np.float32)
    bl = np.asarray(bl, np.float32)
    cw = np.asarray(cw, np.float32)
    cb = np.asarray(cb, np.float32)

    w_emb = Wl[:DIM, 0]
    w_d3 = Wl[DIM:, 0]

    u = np.zeros((DIM, 8), np.float32)
    u[:, 0] = 1.0                      # s = x @ ones
    c1 = np.zeros(2, np.float32)
    c2 = np.zeros(2, np.float32)
    c0 = np.zeros(2, np.float32)
    for i in range(2):
        cw2 = cw[i, 2]
        cw12 = cw[i, 1] * cw2
        cw012 = cw[i, 0] * cw12
        u[:, 1 + 3 * i] = cw2 * w_emb
        u[:, 2 + 3 * i] = cw12 * w_emb
        u[:, 3 + 3 * i] = cw012 * w_emb
        c1[i] = float(np.dot(cb[i, 1] * cw2, w_emb))
        c2[i] = float(np.dot(cb[i, 0] * cw12, w_emb))
        c0[i] = float(np.dot(cb[i, 2], w_emb))
    u[:, 7] = w_emb                    # y0 = x @ w_emb

    w1aug = np.zeros((DIM, 512), np.float32)
    w1aug[:, 0:480] = W1[:, 0:480]
    w1aug[:, 480:488] = u
    w1aug[:, 488:508] = W1[:, 480:500]

    b1full = np.zeros(512, np.float32)
    b1full[0:480] = b1[0:480]
    b1full[480:488] = [0.0, c1[0], c2[0], 0.0, c1[1], c2[1], 0.0, 0.0]
    b1full[488:508] = b1[480:500]
    b1aug = np.ascontiguousarray(b1full.reshape(4, 128).T)

    w2aug = np.zeros((512, H2), np.float32)
    w2aug[0:480] = W2[0:480]
    w2aug[488:508] = W2[480:500]

    sel = np.zeros((128, 2), np.float32)
    sel[97:100, 0] = 1.0
    sel[103, 0] = 1.0
    sel[100:103, 1] = 1.0
    sel[103, 1] = 1.0

    wd3dup = np.ascontiguousarray(np.stack([w_d3, w_d3], axis=1))
    b2arr = np.ascontiguousarray(b2.reshape(2, 100).T)
    b3arr = np.ascontiguousarray(b3.reshape(2, 100).T)
    sigb = np.array([[c0[0] + bl[0]], [c0[1] + bl[0]]], np.float32)

    # tail-round masks: round j multiplies tail row r by
    # (mask_one[j][r] + mask_s[j][r]*s); after 3 rounds the rows
    # [s, y1_0, y2_0, y3_0, y1_1, y2_1, y3_1, y0] carry [s, y1*s, y2*s^2,
    # y3*s^3, ..., y0].  tailmask[:, j] = mask_s, tailmask[:, 3+j] = mask_one.
    tailmask = np.zeros((128, 6), np.float32)
    ones_masks = [[1, 0, 0, 0, 0, 0, 0, 1],
                  [1, 1, 0, 0, 1, 0, 0, 1],
                  [1, 1, 1, 0, 1, 1, 0, 1]]
    s_masks = [[0, 1, 1, 1, 1, 1, 1, 0],
               [0, 0, 1, 1, 0, 1, 1, 0],
               [0, 0, 0, 1, 0, 0, 1, 0]]
    for j in range(3):
        tailmask[96:104, j] = s_masks[j]
        tailmask[96:104, 3 + j] = ones_masks[j]

    w1aug = w1aug.astype(BF16)

    return dict(w1aug=w1aug, w2aug=w2aug, w3m=np.ascontiguousarray(W3),
                wd3dup=wd3dup, sel=sel, b1aug=b1aug, b2arr=b2arr,
                b3arr=b3arr, sigb=sigb, tailmask=tailmask,
                onesrow=np.ones((1, NTILE), np.float32))


def _make_runner(nc, n_cores):
    """Cached jitted shard_map executor for a prebuilt Bass module
    (same lowering path as bass2jax.run_bass_via_pjrt, but reusable
    across calls so repeat invocations skip retrace/recompile)."""
    import jax
    import concourse.mybir as mybir
    from jax.sharding import Mesh, PartitionSpec
    from jax.experimental.shard_map import shard_map
    from concourse.bass2jax import (_bass_exec_p, install_neuronx_cc_hook,
                                    partition_id_tensor)

    install_neuronx_cc_hook()
    partition_name = nc.partition_id_tensor.name if nc.partition_id_tensor else None
    in_names, out_names, out_avals, zero_outs = [], [], [], []
    for alloc in nc.m.functions[0].allocations:
        if not isinstance(alloc, mybir.MemoryLocationSet):
            continue
        name = alloc.memorylocations[0].name
        if alloc.kind == "ExternalInput":
            if name != partition_name:
                in_names.append(name)
        elif alloc.kind == "ExternalOutput":
            out_names.append(name)
            shape = tuple(alloc.tensor_shape)
            dtype = mybir.dt.np(alloc.dtype)
            out_avals.append(jax.core.ShapedArray(shape, dtype))
            zero_outs.append(np.zeros(shape, dtype))
    n_params = len(in_names)
    n_outs = len(out_avals)
    all_in_names = list(in_names) + out_names
    if partition_name is not None:
        all_in_names.append(partition_name)
    donate = tuple(range(n_params, n_params + n_outs))

    def _body(*args):
        operands = list(args)
        if partition_name is not None:
            operands.append(partition_id_tensor())
        outs = _bass_exec_p.bind(
            *operands,
            out_avals=tuple(out_avals),
            in_names=tuple(all_in_names),
            out_names=tuple(out_names),
            lowering_input_output_aliases=(),
            sim_require_finite=True,
            sim_require_nnan=True,
            nc=nc,
        )
        return tuple(outs)

    devices = jax.devices()[:n_cores]
    mesh = Mesh(np.asarray(devices), ("core",))
    in_specs = (PartitionSpec("core"),) * (n_params + n_outs)
    out_specs = (PartitionSpec("core"),) * len(out_names)
    sharded = jax.jit(
        shard_map(_body, mesh=mesh, in_specs=in_specs, out_specs=out_specs,
                  check_rep=False),
        donate_argnums=donate, keep_unused=True)
    return dict(fn=sharded, in_names=in_names, out_names=out_names,
                zero_outs=zero_outs, mesh=mesh)


def kernel(x, show_index, st, W1, b1, W2, b2, W3, b3, Wl, bl, cw, cb):
    x_bf = np.asarray(x, np.float32).astype(BF16)
    # per-core pre-tiled transposed shards, stacked:
    # [(core, n, f, p), c] with block (n,f) = x^T[f*128:+128, n*512:+512]
    xt_all = np.ascontiguousarray(
        x_bf.reshape(NCORES, NT, NTILE, KF, 128)
            .transpose(0, 1, 3, 4, 2)
            .reshape(NCORES * NT * KF * 128, NTILE))
    params = _prep_host(W1, b1, W2, b2, W3, b3, Wl, bl, cw, cb)

    if "runner" not in _CACHE:
        nc = _build_nc()
        _CACHE["nc"] = nc
        _CACHE["runner"] = _make_runner(nc, NCORES)
    r = _CACHE["runner"]

    arrs = {"xt_shard": xt_all}
    for k, v in params.items():
        arrs[k] = np.concatenate([v] * NCORES, axis=0)
    concat_in = [arrs[n] for n in r["in_names"]]
    concat_zeros = [np.zeros((NCORES * z.shape[0], *z.shape[1:]), z.dtype)
                    for z in r["zero_outs"]]
    outs = r["fn"](*concat_in, *concat_zeros)
    logits = np.asarray(outs[0]).reshape(NCORES, 2, BPC).astype(np.float32)
    preds = 1.0 / (1.0 + np.exp(-logits))

    p0 = np.concatenate([preds[c, 0] for c in range(NCORES)]).reshape(B, 1)
    p1 = np.concatenate([preds[c, 1] for c in range(NCORES)]).reshape(B, 1)
    return (p0.astype(np.float32), p1.astype(np.float32))
